# revision 14
# baseline (speedup 1.0000x reference)
"""Multi-head attention (B=2, S=2048, H=1024, 16 heads) on 8 TRN2 NeuronCores.

Sharding (tensor-parallel heads x data-parallel batch, per the hint):
  core c -> batch b = c // 4, head group g = c % 4 (4 heads each).

v2 design (vs the v1 transpose-based kernel):
  - V is computed in natural orientation directly (stationary = x token
    chunks, moving = Wv) -- no PE transpose pass, no strided DVE unpack.
  - Attention runs head-major: per head, scores+exp for all four 512-token
    q-blocks stream through two PSUM score tiles while ctx accumulates into
    FOUR per-block PSUM banks, k-chunk-major, so each V-strip LDWEIGHTS is
    reused by 4 matmuls and consecutive ctx matmuls never share a bank
    (kills the v1 ctxA/ctxB merge pass entirely).
  - exp is split across two engines: ~5/8 of chunk-pairs on ScalarE
    (table exp, scale=1/ALPHA bias=-CSHIFT), ~3/8 on VectorE via a
    Schraudolph bf16 exp: probs_bits = int16(alpha*s + beta), where
    alpha = 128*log2(e) is folded into Wq on the host, so the DVE op is a
    single tensor_scalar(add beta, max 0) with int16 output bitcast onto
    the bf16 probs tile.  The softmax denominator (the M=65 ones-column in
    the ctx stationary) is built from the same approximate probs, so the
    systematic Schraudolph error largely cancels in the division.
  - Out-projection runs as a tail after the last division, with the
    PSUM->SBUF copies split between ScalarE and VectorE.

Numerics: fp16 projections (f32 PSUM accum), fp16 scores operands, bf16
probs/V/ctx.  Full-model emulation of this scheme measures rel_err ~9.5e-3
(gate is 2e-2).  bq/bk applied on device (scaled by ALPHA for bq);
bv/bo folded into a host-side additive constant (exact).
"""

import ml_dtypes
import numpy as np

import concourse.bacc as bacc
import concourse.mybir as mybir
import concourse.tile as tile
from concourse.bass_utils import run_bass_kernel_spmd

NCORES = 8
B, S, HID = 2, 2048, 1024
NH, HD = 16, 64
HPC = 4            # heads per core
QC = HPC * HD      # 256 local projection cols per core
HC = HID // 128    # 8 hidden chunks
TC = S // 128      # 16 token chunks
TB = S // 512      # 4 token blocks

ALPHA = 128.0 * np.log2(np.e)                    # 184.6637 folded into Wq
CSHIFT = 2.0                                     # probs scaled by e^-CSHIFT
BETA = 16256.0 - CSHIFT * ALPHA + 0.5 - 5.57     # bias + trunc comp + centering
VSTRIDE = HPC * (HD + 1)                         # 260: v_sb cols per token chunk

F32 = mybir.dt.float32
BF16 = mybir.dt.bfloat16
FP16 = mybir.dt.float16
I16 = mybir.dt.int16
EXP = mybir.ActivationFunctionType.Exp
MULT = mybir.AluOpType.mult
ADD = mybir.AluOpType.add
MAX = mybir.AluOpType.max

# chunk-pair slot -> True if exp goes to the DVE Schraudolph path (6/16
# per half-pass = 37.5%)
def _on_dve(h, cp, u):
    return (cp * 2 + u + h) % 8 in (2, 5, 7)


def build_nc():
    nc = bacc.Bacc("TRN2", target_bir_lowering=False, debug=False,
                   num_devices=NCORES)
    xT = nc.declare_dram_parameter("xT", [HID, S], FP16, isOutput=False)
    wq = nc.declare_dram_parameter("wq", [HID, QC], FP16, isOutput=False)
    wk = nc.declare_dram_parameter("wk", [HID, QC], FP16, isOutput=False)
    wv = nc.declare_dram_parameter("wv", [HID, QC], FP16, isOutput=False)
    wo = nc.declare_dram_parameter("wo", [QC, HID], BF16, isOutput=False)
    bq = nc.declare_dram_parameter("bq", [QC], F32, isOutput=False)
    bk = nc.declare_dram_parameter("bk", [QC], F32, isOutput=False)
    out = nc.declare_dram_parameter("out", [S, HID], BF16, isOutput=True)

    with tile.TileContext(nc) as tc:
        with (
            tc.tile_pool(name="const", bufs=1) as constp,
            tc.tile_pool(name="qkv", bufs=1) as qkvp,
        ):
            wo_sb = constp.tile([128, 2 * HID], BF16)
            bq_sb = constp.tile([128, 2], F32)
            bk_sb = constp.tile([128, 2], F32)
            warm = constp.tile([1, 8], F32)
            nbias = constp.tile([128, 1], F32)
            nc.vector.memset(nbias[:], -CSHIFT)
            # Q^T/K^T per head, duplicated across both partition halves so the
            # scores matmuls run two tok_k chunks concurrently as row-tiles.
            qt2 = qkvp.tile([128, HPC * S], FP16)
            kt2 = qkvp.tile([128, HPC * S], FP16)
            # Natural V (bf16): per token chunk t, 4 strips [128, 65]
            # (64 v cols + a ones col at 64 -> softmax denominator lands in
            # ctx row 64 of the same accumulation).
            v_sb = qkvp.tile([128, TC * VSTRIDE], BF16)
            ctxf_sb = qkvp.tile([128, 2 * S], BF16)

            # preload the exp table set while DMAs run
            nc.vector.memset(warm[:], 0.0)
            nc.scalar.activation(warm[:], warm[:], EXP)
            # ones columns: set everything to 1.0, V copies overwrite cols 0:64
            nc.vector.memset(v_sb[:], 1.0)

            # ---- phase 1: projections -------------------------------------
            with tc.tile_pool(name="xw", bufs=1) as xwp:
                xT_sb = xwp.tile([128, HC * S], FP16)
                wq_sb = xwp.tile([128, HC * QC], FP16)
                wk_sb = xwp.tile([128, HC * QC], FP16)
                wv_sb = xwp.tile([128, HC * QC], FP16)

                # wv first, then xT in j-quarters so V chunk 0 starts asap
                for hc in range(HC):
                    r = slice(hc * 128, (hc + 1) * 128)
                    eng = nc.scalar if hc % 2 == 0 else nc.sync
                    eng.dma_start(wv_sb[:, hc * QC:(hc + 1) * QC], wv[r, :])
                xt_dmas = {}
                for jq in range(TB):
                    for hc in range(HC):
                        r = slice(hc * 128, (hc + 1) * 128)
                        eng = nc.sync if hc % 2 == 0 else nc.scalar
                        xt_dmas[(jq, hc)] = eng.dma_start(
                            xT_sb[:, hc * S + jq * 512:hc * S + (jq + 1) * 512],
                            xT[r, jq * 512:(jq + 1) * 512])
                for ci in range(2):
                    nc.sync.dma_start(bq_sb[:, ci:ci + 1],
                                      bq[ci * 128:(ci + 1) * 128])
                    nc.sync.dma_start(bk_sb[:, ci:ci + 1],
                                      bk[ci * 128:(ci + 1) * 128])
                qk_dmas = []
                for hc in range(HC):
                    r = slice(hc * 128, (hc + 1) * 128)
                    qk_dmas.append(nc.sync.dma_start(
                        wq_sb[:, hc * QC:(hc + 1) * QC], wq[r, :]))
                    qk_dmas.append(nc.scalar.dma_start(
                        wk_sb[:, hc * QC:(hc + 1) * QC], wk[r, :]))

                # V natural: stationary = x token chunk, moving = Wv
                v_mms = {}
                with tc.tile_pool(name="vps", bufs=4, space="PSUM") as vps:
                    for t in range(TC):
                        vp = vps.tile([128, QC], F32, tag="vps")
                        for hc in range(HC):
                            mm = nc.tensor.matmul(
                                vp[:, :],
                                xT_sb[:, hc * S + t * 128:hc * S + t * 128 + 128],
                                wv_sb[:, hc * QC:(hc + 1) * QC],
                                start=(hc == 0), stop=(hc == HC - 1))
                            v_mms[(t, hc)] = mm
                        dst = v_sb[:, t * VSTRIDE:(t + 1) * VSTRIDE].rearrange(
                            "p (h e) -> p h e", h=HPC)[:, :, 0:HD]
                        src = vp[:, :].rearrange("p (h e) -> p h e", h=HPC)
                        nc.vector.tensor_copy(dst, src)

                # pace the w / wo input loads behind early V matmuls
                for i, d in enumerate(qk_dmas):
                    src_mm = v_mms[(min(2 + i // 2, TC - 1), 0)]
                    tile.add_dep_helper(d.ins, src_mm.ins,
                                        reason="pace w input load")
                for ci in range(2):
                    d = nc.gpsimd.dma_start(
                        wo_sb[:, ci * HID:(ci + 1) * HID],
                        wo[ci * 128:(ci + 1) * 128, :])
                    tile.add_dep_helper(d.ins, v_mms[(10 + 2 * ci, 0)].ins,
                                        reason="pace wo load")

                # Q^T and K^T (psum [128, S] per (proj, ci)), written into the
                # duplicated per-head layout.  ALPHA is folded into wq/bq.
                with tc.tile_pool(name="ps1", bufs=2, space="PSUM") as ps1:
                    for ci in range(2):
                        for w_sb, b_sb, dst in ((wq_sb, bq_sb, qt2),
                                                (wk_sb, bk_sb, kt2)):
                            ps = ps1.tile([128, S], F32, tag="ps1")
                            for hc in range(HC):
                                for j in range(TB):
                                    nc.tensor.matmul(
                                        ps[:, j * 512:(j + 1) * 512],
                                        w_sb[:, hc * QC + ci * 128:
                                             hc * QC + ci * 128 + 128],
                                        xT_sb[:, hc * S + j * 512:
                                              hc * S + j * 512 + 512],
                                        start=(hc == 0), stop=(hc == HC - 1))
                            hA, hB = 2 * ci, 2 * ci + 1
                            nc.vector.tensor_scalar_add(
                                dst[0:64, hA * S:(hA + 1) * S], ps[0:64, :],
                                b_sb[0:64, ci:ci + 1])
                            nc.scalar.activation(
                                dst[64:128, hB * S:(hB + 1) * S], ps[64:128, :],
                                mybir.ActivationFunctionType.Identity,
                                bias=b_sb[64:128, ci:ci + 1])
                            nc.sync.dma_start(dst[64:128, hA * S:(hA + 1) * S],
                                              dst[0:64, hA * S:(hA + 1) * S])
                            nc.scalar.dma_start(dst[0:64, hB * S:(hB + 1) * S],
                                                dst[64:128, hB * S:(hB + 1) * S])

            # ---- phase 2: attention, head-major in 2-block half-passes ----
            # Per (head, jp) half-pass: 8 score slots (a k-chunk pair vs both
            # q-blocks, sharing the two kt2 stationaries) pipelined one slot
            # ahead of ctx, which accumulates k-chunk-major into two PSUM
            # banks (one per q-block, so consecutive ctx matmuls never share
            # a bank).  PSUM: scores 3x2 + ctx 2 = 8 banks.
            with (
                tc.tile_pool(name="probs", bufs=1) as probsp,
                tc.tile_pool(name="div", bufs=4) as divp,
                tc.tile_pool(name="scps", bufs=3, space="PSUM") as scps,
                tc.tile_pool(name="ctps", bufs=1, space="PSUM") as ctps,
            ):
                probs = [probsp.tile([128, 8 * 1024], BF16, tag=f"probs{u}",
                                     name=f"probs{u}")
                         for u in range(2)]

                def emit_scores_slot(h, jp, cp):
                    """scores + exp for chunk pair (2cp, 2cp+1), 2 q-blocks."""
                    hS = h * S
                    c0, c1 = 2 * cp, 2 * cp + 1
                    for u in range(2):
                        j = 2 * jp + u
                        sp = scps.tile([128, 1024], F32, tag="sc")
                        nc.tensor.matmul(
                            sp[:, 0:512],
                            kt2[0:64, hS + c0 * 128:hS + c0 * 128 + 128],
                            qt2[0:64, hS + j * 512:hS + j * 512 + 512],
                            start=True, stop=True, tile_position=(0, 0))
                        nc.tensor.matmul(
                            sp[:, 512:1024],
                            kt2[64:128, hS + c1 * 128:hS + c1 * 128 + 128],
                            qt2[64:128, hS + j * 512:hS + j * 512 + 512],
                            start=True, stop=True, tile_position=(64, 0))
                        dst = probs[u][:, cp * 1024:(cp + 1) * 1024]
                        if _on_dve(h, cp, u):
                            nc.vector.tensor_scalar(
                                out=dst.bitcast(I16), in0=sp[:, :],
                                scalar1=BETA, scalar2=0.0, op0=ADD, op1=MAX)
                        else:
                            nc.scalar.activation(dst, sp[:, :], EXP,
                                                 bias=nbias[:, 0:1],
                                                 scale=1.0 / ALPHA)

                ctx_map = {}
                last_ctx = [None]

                def emit_ctx_pair(h, jp, cp):
                    """ctx chunk-pair: per chunk, both q-block matmuls."""
                    for c in (2 * cp, 2 * cp + 1):
                        strip = v_sb[:, c * VSTRIDE + h * (HD + 1):
                                     c * VSTRIDE + h * (HD + 1) + HD + 1]
                        for u in range(2):
                            if cp == 0 and c == 0:
                                ctx_map[(h, 2 * jp + u)] = ctps.tile(
                                    [128, 512], F32, tag=f"ctx{u}",
                                    name=f"ctx_{h}_{2 * jp + u}")
                            mm = nc.tensor.matmul(
                                ctx_map[(h, 2 * jp + u)][0:HD + 1, :],
                                strip,
                                probs[u][:, c * 512:(c + 1) * 512],
                                start=(c == 0), stop=(c == TC - 1))
                            # keep the scheduler from pairing same-bank MMs
                            if last_ctx[0] is not None:
                                tile.add_dep_helper(mm.ins, last_ctx[0].ins,
                                                    reason="ctx order")
                            last_ctx[0] = mm

                def emit_division(h, j):
                    ci, lo = h // 2, (h % 2) * 64
                    craw = ctx_map.pop((h, j))
                    dn = divp.tile([65, 512], F32, tag="dn")
                    nc.vector.tensor_copy(dn[64:65, :], craw[64:65, :])
                    denr = divp.tile([128, 4], F32, tag="denr")
                    nc.sync.dma_start(denr[:, :], dn[64:65, :])
                    recr = divp.tile([128, 4], F32, tag="recr")
                    nc.vector.reciprocal(recr[:, :], denr[:, :])
                    rrow = divp.tile([1, 512], F32, tag="rrow")
                    nc.sync.dma_start(rrow[:, :], recr[:, :])
                    Dt = divp.tile([64, 512], F32, tag="Dt")
                    nc.gpsimd.partition_broadcast(Dt[:, :], rrow[0:1, :])
                    o = ci * S + j * 512
                    if lo == 0:
                        nc.vector.tensor_tensor(
                            out=ctxf_sb[0:64, o:o + 512],
                            in0=craw[0:64, :], in1=Dt[:, :], op=MULT)
                    else:
                        ctxd = divp.tile([64, 512], BF16, tag="ctxd")
                        nc.vector.tensor_tensor(
                            out=ctxd[:, :], in0=craw[0:64, :],
                            in1=Dt[:, :], op=MULT)
                        nc.gpsimd.dma_start(ctxf_sb[64:128, o:o + 512],
                                            ctxd[:, :])

                for h in range(HPC):
                    for jp in range(2):
                        emit_scores_slot(h, jp, 0)
                        for cp in range(1, TC // 2):
                            emit_scores_slot(h, jp, cp)
                            emit_ctx_pair(h, jp, cp - 1)
                        emit_ctx_pair(h, jp, TC // 2 - 1)
                        emit_division(h, 2 * jp)
                        emit_division(h, 2 * jp + 1)

            # ---- phase 3: out-projection tail -----------------------------
            with (
                tc.tile_pool(name="ostg", bufs=3) as ostg,
                tc.tile_pool(name="ops", bufs=2, space="PSUM") as ops,
            ):
                for t in range(TC):
                    ot = ostg.tile([128, 1024], BF16, tag="ot")
                    op0 = ops.tile([128, 512], F32, tag="op0")
                    op1 = ops.tile([128, 512], F32, tag="op1")
                    # ci-major so each ctxf stationary load serves 2 matmuls
                    # and consecutive matmuls alternate PSUM banks
                    for ci in range(2):
                        for oc, op in ((0, op0), (1, op1)):
                            nc.tensor.matmul(
                                op[:, :],
                                ctxf_sb[:, ci * S + t * 128:ci * S + t * 128 + 128],
                                wo_sb[:, ci * HID + oc * 512:
                                      ci * HID + oc * 512 + 512],
                                start=(ci == 0), stop=(ci == 1))
                    nc.vector.tensor_copy(ot[:, 0:512], op0[:, :])
                    nc.scalar.copy(ot[:, 512:1024], op1[:, :])
                    nc.sync.dma_start(out[t * 128:(t + 1) * 128, :], ot[:, :])

    nc.compile()
    return nc


_NC = None


def _get_nc():
    global _NC
    if _NC is None:
        _NC = build_nc()
    return _NC


def make_in_maps(x, Wq, bq, Wk, bk, Wv, bv, Wo, bo):
    qscale = 0.125 * ALPHA
    in_maps = []
    for core in range(NCORES):
        b, g = core // 4, core % 4
        sl = slice(g * QC, (g + 1) * QC)
        in_maps.append({
            "xT": np.ascontiguousarray(x[b].T).astype(np.float16),
            "wq": (np.ascontiguousarray(Wq[:, sl]) * qscale).astype(np.float16),
            "wk": np.ascontiguousarray(Wk[:, sl]).astype(np.float16),
            "wv": np.ascontiguousarray(Wv[:, sl]).astype(np.float16),
            "wo": np.ascontiguousarray(Wo[sl, :]).astype(ml_dtypes.bfloat16),
            "bq": (np.asarray(bq[sl]) * qscale).astype(np.float32),
            "bk": np.asarray(bk[sl]).astype(np.float32),
        })
    return in_maps


def combine_outputs(core_outs, Wv_bias_term):
    full = np.empty((B, S, HID), np.float32)
    for b in range(B):
        acc = core_outs[4 * b].astype(np.float32).copy()
        for g in range(1, 4):
            acc += core_outs[4 * b + g]
        full[b] = acc + Wv_bias_term
    return full


def kernel(**inputs):
    x = np.asarray(inputs["x"], np.float32)
    Wq = np.asarray(inputs["Wq"], np.float32)
    bq = np.asarray(inputs["bq"], np.float32)
    Wk = np.asarray(inputs["Wk"], np.float32)
    bk = np.asarray(inputs["bk"], np.float32)
    Wv = np.asarray(inputs["Wv"], np.float32)
    bv = np.asarray(inputs["bv"], np.float32)
    Wo = np.asarray(inputs["Wo"], np.float32)
    bo = np.asarray(inputs["bo"], np.float32)

    nc = _get_nc()
    in_maps = make_in_maps(x, Wq, bq, Wk, bk, Wv, bv, Wo, bo)
    res = run_bass_kernel_spmd(nc, in_maps, core_ids=list(range(NCORES)))
    core_outs = [res.results[c]["out"] for c in range(NCORES)]
    bias_term = (bv @ Wo + bo).astype(np.float32)
    return combine_outputs(core_outs, bias_term)


# revision 21
# speedup vs baseline: 1.2714x; 1.2714x over previous
"""Multi-head attention (B=2, S=2048, H=1024, 16 heads) on 8 TRN2 NeuronCores.

Sharding (tensor-parallel heads x data-parallel batch, per the hint):
  core c -> batch b = c // 4, head group g = c % 4 (4 heads each).

v2 design (vs the v1 transpose-based kernel):
  - V is computed in natural orientation directly (stationary = x token
    chunks, moving = Wv) -- no PE transpose pass, no strided DVE unpack.
  - Attention runs head-major: per head, scores+exp for all four 512-token
    q-blocks stream through two PSUM score tiles while ctx accumulates into
    FOUR per-block PSUM banks, k-chunk-major, so each V-strip LDWEIGHTS is
    reused by 4 matmuls and consecutive ctx matmuls never share a bank
    (kills the v1 ctxA/ctxB merge pass entirely).
  - exp is split across two engines: ~5/8 of chunk-pairs on ScalarE
    (table exp, scale=1/ALPHA bias=-CSHIFT), ~3/8 on VectorE via a
    Schraudolph bf16 exp: probs_bits = int16(alpha*s + beta), where
    alpha = 128*log2(e) is folded into Wq on the host, so the DVE op is a
    single tensor_scalar(add beta, max 0) with int16 output bitcast onto
    the bf16 probs tile.  The softmax denominator (the M=65 ones-column in
    the ctx stationary) is built from the same approximate probs, so the
    systematic Schraudolph error largely cancels in the division.
  - Out-projection runs as a tail after the last division, with the
    PSUM->SBUF copies split between ScalarE and VectorE.

Numerics: fp16 projections (f32 PSUM accum), fp16 scores operands, bf16
probs/V/ctx.  Full-model emulation of this scheme measures rel_err ~9.5e-3
(gate is 2e-2).  bq/bk applied on device (scaled by ALPHA for bq);
bv/bo folded into a host-side additive constant (exact).
"""

import ml_dtypes
import numpy as np

import concourse.bacc as bacc
import concourse.mybir as mybir
import concourse.tile as tile
from concourse.bass_utils import run_bass_kernel_spmd

NCORES = 8
B, S, HID = 2, 2048, 1024
NH, HD = 16, 64
HPC = 4            # heads per core
QC = HPC * HD      # 256 local projection cols per core
HC = HID // 128    # 8 hidden chunks
TC = S // 128      # 16 token chunks
TB = S // 512      # 4 token blocks

ALPHA = 128.0 * np.log2(np.e)                    # 184.6637 folded into Wq
CSHIFT = 2.0                                     # probs scaled by e^-CSHIFT
BETA = 16256.0 - CSHIFT * ALPHA + 0.5 - 5.57     # bias + trunc comp + centering
VSTRIDE = HPC * (HD + 1)                         # 260: v_sb cols per token chunk

F32 = mybir.dt.float32
BF16 = mybir.dt.bfloat16
FP16 = mybir.dt.float16
I16 = mybir.dt.int16
EXP = mybir.ActivationFunctionType.Exp
MULT = mybir.AluOpType.mult
ADD = mybir.AluOpType.add
MAX = mybir.AluOpType.max

# chunk-pair slot -> True if exp goes to the DVE Schraudolph path (6/16
# per half-pass = 37.5%)
def _on_dve(h, cp, u):
    return (cp * 2 + u + h) % 8 in (2, 5, 7)


def build_nc():
    nc = bacc.Bacc("TRN2", target_bir_lowering=False, debug=False,
                   num_devices=NCORES)
    xT = nc.declare_dram_parameter("xT", [HID, S], FP16, isOutput=False)
    wq = nc.declare_dram_parameter("wq", [HID, QC], FP16, isOutput=False)
    wk = nc.declare_dram_parameter("wk", [HID, QC], FP16, isOutput=False)
    wv = nc.declare_dram_parameter("wv", [HID, QC], FP16, isOutput=False)
    wo = nc.declare_dram_parameter("wo", [QC, HID], BF16, isOutput=False)
    bq = nc.declare_dram_parameter("bq", [QC], F32, isOutput=False)
    bk = nc.declare_dram_parameter("bk", [QC], F32, isOutput=False)
    out = nc.declare_dram_parameter("out", [S, HID], BF16, isOutput=True)

    with tile.TileContext(nc) as tc:
        with (
            tc.tile_pool(name="const", bufs=1) as constp,
            tc.tile_pool(name="qkv", bufs=1) as qkvp,
        ):
            wo_sb = constp.tile([128, 2 * HID], BF16)
            bq_sb = constp.tile([128, 2], F32)
            bk_sb = constp.tile([128, 2], F32)
            warm = constp.tile([1, 8], F32)
            nbias = constp.tile([128, 1], F32)
            nc.vector.memset(nbias[:], -CSHIFT)
            # Q^T/K^T per head, duplicated across both partition halves so the
            # scores matmuls run two tok_k chunks concurrently as row-tiles.
            qt2 = qkvp.tile([128, HPC * S], FP16)
            kt2 = qkvp.tile([128, HPC * S], FP16)
            # Natural V (bf16): per token chunk t, 4 strips [128, 65]
            # (64 v cols + a ones col at 64 -> softmax denominator lands in
            # ctx row 64 of the same accumulation).
            v_sb = qkvp.tile([128, TC * VSTRIDE], BF16)
            ctxf_sb = qkvp.tile([128, 2 * S], BF16)

            # preload the exp table set while DMAs run
            nc.vector.memset(warm[:], 0.0)
            nc.scalar.activation(warm[:], warm[:], EXP)
            # ones columns: set everything to 1.0, V copies overwrite cols 0:64
            nc.vector.memset(v_sb[:], 1.0)

            # ---- phase 1: projections -------------------------------------
            with tc.tile_pool(name="xw", bufs=1) as xwp:
                xT_sb = xwp.tile([128, HC * S], FP16)
                wq_sb = xwp.tile([128, HC * QC], FP16)
                wk_sb = xwp.tile([128, HC * QC], FP16)
                wv_sb = xwp.tile([128, HC * QC], FP16)

                # wv first, then xT in j-quarters so V chunk 0 starts asap
                for hc in range(HC):
                    r = slice(hc * 128, (hc + 1) * 128)
                    eng = nc.scalar if hc % 2 == 0 else nc.sync
                    eng.dma_start(wv_sb[:, hc * QC:(hc + 1) * QC], wv[r, :])
                xt_dmas = {}
                for jq in range(TB):
                    for hc in range(HC):
                        r = slice(hc * 128, (hc + 1) * 128)
                        eng = nc.sync if hc % 2 == 0 else nc.scalar
                        xt_dmas[(jq, hc)] = eng.dma_start(
                            xT_sb[:, hc * S + jq * 512:hc * S + (jq + 1) * 512],
                            xT[r, jq * 512:(jq + 1) * 512])
                for ci in range(2):
                    nc.sync.dma_start(bq_sb[:, ci:ci + 1],
                                      bq[ci * 128:(ci + 1) * 128])
                    nc.sync.dma_start(bk_sb[:, ci:ci + 1],
                                      bk[ci * 128:(ci + 1) * 128])
                qk_dmas = []
                for hc in range(HC):
                    r = slice(hc * 128, (hc + 1) * 128)
                    qk_dmas.append(nc.sync.dma_start(
                        wq_sb[:, hc * QC:(hc + 1) * QC], wq[r, :]))
                    qk_dmas.append(nc.scalar.dma_start(
                        wk_sb[:, hc * QC:(hc + 1) * QC], wk[r, :]))

                # V natural: stationary = x token chunk, moving = Wv
                v_mms = {}
                with tc.tile_pool(name="vps", bufs=4, space="PSUM") as vps:
                    for t in range(TC):
                        vp = vps.tile([128, QC], F32, tag="vps")
                        for hc in range(HC):
                            mm = nc.tensor.matmul(
                                vp[:, :],
                                xT_sb[:, hc * S + t * 128:hc * S + t * 128 + 128],
                                wv_sb[:, hc * QC:(hc + 1) * QC],
                                start=(hc == 0), stop=(hc == HC - 1))
                            v_mms[(t, hc)] = mm
                        dst = v_sb[:, t * VSTRIDE:(t + 1) * VSTRIDE].rearrange(
                            "p (h e) -> p h e", h=HPC)[:, :, 1:HD + 1]
                        src = vp[:, :].rearrange("p (h e) -> p h e", h=HPC)
                        nc.vector.tensor_copy(dst, src)

                # pace the w / wo input loads behind early V matmuls
                for i, d in enumerate(qk_dmas):
                    src_mm = v_mms[(min(2 + i // 2, TC - 1), 0)]
                    tile.add_dep_helper(d.ins, src_mm.ins,
                                        reason="pace w input load")
                for ci in range(2):
                    d = nc.gpsimd.dma_start(
                        wo_sb[:, ci * HID:(ci + 1) * HID],
                        wo[ci * 128:(ci + 1) * 128, :])
                    tile.add_dep_helper(d.ins, v_mms[(10 + 2 * ci, 0)].ins,
                                        reason="pace wo load")

                # Q^T and K^T (psum [128, S] per (proj, ci)), written into the
                # duplicated per-head layout.  ALPHA is folded into wq/bq.
                with tc.tile_pool(name="ps1", bufs=2, space="PSUM") as ps1:
                    for ci in range(2):
                        for w_sb, b_sb, dst in ((wq_sb, bq_sb, qt2),
                                                (wk_sb, bk_sb, kt2)):
                            ps = ps1.tile([128, S], F32, tag="ps1")
                            for hc in range(HC):
                                for j in range(TB):
                                    nc.tensor.matmul(
                                        ps[:, j * 512:(j + 1) * 512],
                                        w_sb[:, hc * QC + ci * 128:
                                             hc * QC + ci * 128 + 128],
                                        xT_sb[:, hc * S + j * 512:
                                              hc * S + j * 512 + 512],
                                        start=(hc == 0), stop=(hc == HC - 1))
                            hA, hB = 2 * ci, 2 * ci + 1
                            nc.vector.tensor_scalar_add(
                                dst[0:64, hA * S:(hA + 1) * S], ps[0:64, :],
                                b_sb[0:64, ci:ci + 1])
                            nc.scalar.activation(
                                dst[64:128, hB * S:(hB + 1) * S], ps[64:128, :],
                                mybir.ActivationFunctionType.Identity,
                                bias=b_sb[64:128, ci:ci + 1])
                            nc.sync.dma_start(dst[64:128, hA * S:(hA + 1) * S],
                                              dst[0:64, hA * S:(hA + 1) * S])
                            nc.scalar.dma_start(dst[0:64, hB * S:(hB + 1) * S],
                                                dst[64:128, hB * S:(hB + 1) * S])

            # ---- phase 2: attention, head-major ---------------------------
            # Per head: score slots (a k-chunk pair vs all 4 q-blocks,
            # sharing the two kt2 stationaries) pipelined one slot ahead of
            # ctx, which accumulates k-chunk-major into four per-q-block PSUM
            # banks.  The softmax denominator rides in ctx row 0 (ones column
            # at the front of each V strip), so the division is a PSUM-row-0
            # fast-reciprocal + gpsimd broadcast + one multiply, with no DMA
            # round-trips.  PSUM: scores 2x2 + ctx 4 = 8 banks.
            with (
                tc.tile_pool(name="probs", bufs=1) as probsp,
                tc.tile_pool(name="div", bufs=4) as divp,
                tc.tile_pool(name="scps", bufs=2, space="PSUM") as scps,
                tc.tile_pool(name="ctps", bufs=1, space="PSUM") as ctps,
            ):
                probs = [probsp.tile([128, 8 * 1024], BF16, tag=f"probs{j}",
                                     name=f"probs{j}")
                         for j in range(TB)]

                def emit_scores_slot(h, cp):
                    """scores + exp for chunk pair (2cp, 2cp+1), 4 q-blocks."""
                    hS = h * S
                    c0, c1 = 2 * cp, 2 * cp + 1
                    for j in range(TB):
                        sp = scps.tile([128, 1024], F32, tag="sc")
                        nc.tensor.matmul(
                            sp[:, 0:512],
                            kt2[0:64, hS + c0 * 128:hS + c0 * 128 + 128],
                            qt2[0:64, hS + j * 512:hS + j * 512 + 512],
                            start=True, stop=True, tile_position=(0, 0))
                        nc.tensor.matmul(
                            sp[:, 512:1024],
                            kt2[64:128, hS + c1 * 128:hS + c1 * 128 + 128],
                            qt2[64:128, hS + j * 512:hS + j * 512 + 512],
                            start=True, stop=True, tile_position=(64, 0))
                        dst = probs[j][:, cp * 1024:(cp + 1) * 1024]
                        if _on_dve(h, cp, j):
                            nc.vector.tensor_scalar(
                                out=dst.bitcast(I16), in0=sp[:, :],
                                scalar1=BETA, scalar2=0.0, op0=ADD, op1=MAX)
                        else:
                            nc.scalar.activation(dst, sp[:, :], EXP,
                                                 bias=nbias[:, 0:1],
                                                 scale=1.0 / ALPHA)

                ctx_map = {}

                def emit_ctx_pair(h, cp):
                    """ctx chunk-pair for all 4 q-blocks."""
                    for c in (2 * cp, 2 * cp + 1):
                        strip = v_sb[:, c * VSTRIDE + h * (HD + 1):
                                     c * VSTRIDE + h * (HD + 1) + HD + 1]
                        for j in range(TB):
                            if cp == 0 and c == 0:
                                ctx_map[(h, j)] = ctps.tile(
                                    [128, 512], F32, tag=f"ctx{j}",
                                    name=f"ctx_{h}_{j}")
                            nc.tensor.matmul(
                                ctx_map[(h, j)][0:HD + 1, :],
                                strip,
                                probs[j][:, c * 512:(c + 1) * 512],
                                start=(c == 0), stop=(c == TC - 1))

                def emit_division(h, j):
                    # ctx rows: 0 = softmax denominator, 1..64 = head dims.
                    # The den reciprocal lives at partition 0 so the gpsimd
                    # broadcast root is partition 0 (hw requirement); the
                    # multiply covers all 65 rows (row 0 = den/den, unused)
                    # and the partition shift rides the gpsimd DMA.
                    ci, lo = h // 2, (h % 2) * 64
                    craw = ctx_map.pop((h, j))
                    rcp = divp.tile([1, 512], F32, tag="rcp")
                    nc.vector.reciprocal_approx_fast(rcp[0:1, :],
                                                     craw[0:1, :])
                    Dt = divp.tile([65, 512], F32, tag="Dt")
                    nc.gpsimd.partition_broadcast(Dt[:, :], rcp[0:1, :])
                    ctxd = divp.tile([65, 512], BF16, tag="ctxd")
                    nc.vector.tensor_tensor(
                        out=ctxd[:, :], in0=craw[0:65, :],
                        in1=Dt[:, :], op=MULT)
                    o = ci * S + j * 512
                    nc.gpsimd.dma_start(ctxf_sb[lo:lo + 64, o:o + 512],
                                        ctxd[1:65, :])

                def emit_outproj(j):
                    # out-projection for q-block j, PSUM borrowed from the
                    # (idle by now) scores pool
                    for tt in range(4):
                        t = 4 * j + tt
                        ot = ostg.tile([128, 1024], BF16, tag="ot")
                        op = scps.tile([128, 1024], F32, tag="sc",
                                       name=f"op_{t}")
                        for ci in range(2):
                            for oc in range(2):
                                nc.tensor.matmul(
                                    op[:, oc * 512:(oc + 1) * 512],
                                    ctxf_sb[:, ci * S + t * 128:
                                            ci * S + t * 128 + 128],
                                    wo_sb[:, ci * HID + oc * 512:
                                          ci * HID + oc * 512 + 512],
                                    start=(ci == 0), stop=(ci == 1))
                        nc.vector.tensor_copy(ot[:, 0:512], op[:, 0:512])
                        nc.scalar.copy(ot[:, 512:1024], op[:, 512:1024])
                        nc.sync.dma_start(out[t * 128:(t + 1) * 128, :],
                                          ot[:, :])

                LEAD = 2
                for h in range(HPC):
                    for cp in range(LEAD):
                        emit_scores_slot(h, cp)
                    for cp in range(LEAD, TC // 2):
                        emit_scores_slot(h, cp)
                        emit_ctx_pair(h, cp - LEAD)
                    for cp in range(TC // 2 - LEAD, TC // 2):
                        emit_ctx_pair(h, cp)
                    for j in range(TB):
                        emit_division(h, j)
                        if h == HPC - 1:
                            emit_outproj(j)

            # ---- phase 3: out-projection tail -----------------------------
            with (
                tc.tile_pool(name="ostg", bufs=3) as ostg,
                tc.tile_pool(name="ops", bufs=2, space="PSUM") as ops,
            ):
                for t in range(TC):
                    ot = ostg.tile([128, 1024], BF16, tag="ot")
                    op0 = ops.tile([128, 512], F32, tag="op0")
                    op1 = ops.tile([128, 512], F32, tag="op1")
                    # ci-major so each ctxf stationary load serves 2 matmuls
                    # and consecutive matmuls alternate PSUM banks
                    for ci in range(2):
                        for oc, op in ((0, op0), (1, op1)):
                            nc.tensor.matmul(
                                op[:, :],
                                ctxf_sb[:, ci * S + t * 128:ci * S + t * 128 + 128],
                                wo_sb[:, ci * HID + oc * 512:
                                      ci * HID + oc * 512 + 512],
                                start=(ci == 0), stop=(ci == 1))
                    nc.vector.tensor_copy(ot[:, 0:512], op0[:, :])
                    nc.scalar.copy(ot[:, 512:1024], op1[:, :])
                    nc.sync.dma_start(out[t * 128:(t + 1) * 128, :], ot[:, :])

    nc.compile()
    return nc


_NC = None


def _get_nc():
    global _NC
    if _NC is None:
        _NC = build_nc()
    return _NC


def make_in_maps(x, Wq, bq, Wk, bk, Wv, bv, Wo, bo):
    qscale = 0.125 * ALPHA
    in_maps = []
    for core in range(NCORES):
        b, g = core // 4, core % 4
        sl = slice(g * QC, (g + 1) * QC)
        in_maps.append({
            "xT": np.ascontiguousarray(x[b].T).astype(np.float16),
            "wq": (np.ascontiguousarray(Wq[:, sl]) * qscale).astype(np.float16),
            "wk": np.ascontiguousarray(Wk[:, sl]).astype(np.float16),
            "wv": np.ascontiguousarray(Wv[:, sl]).astype(np.float16),
            "wo": np.ascontiguousarray(Wo[sl, :]).astype(ml_dtypes.bfloat16),
            "bq": (np.asarray(bq[sl]) * qscale).astype(np.float32),
            "bk": np.asarray(bk[sl]).astype(np.float32),
        })
    return in_maps


def combine_outputs(core_outs, Wv_bias_term):
    full = np.empty((B, S, HID), np.float32)
    for b in range(B):
        acc = core_outs[4 * b].astype(np.float32).copy()
        for g in range(1, 4):
            acc += core_outs[4 * b + g]
        full[b] = acc + Wv_bias_term
    return full


def kernel(**inputs):
    x = np.asarray(inputs["x"], np.float32)
    Wq = np.asarray(inputs["Wq"], np.float32)
    bq = np.asarray(inputs["bq"], np.float32)
    Wk = np.asarray(inputs["Wk"], np.float32)
    bk = np.asarray(inputs["bk"], np.float32)
    Wv = np.asarray(inputs["Wv"], np.float32)
    bv = np.asarray(inputs["bv"], np.float32)
    Wo = np.asarray(inputs["Wo"], np.float32)
    bo = np.asarray(inputs["bo"], np.float32)

    nc = _get_nc()
    in_maps = make_in_maps(x, Wq, bq, Wk, bk, Wv, bv, Wo, bo)
    res = run_bass_kernel_spmd(nc, in_maps, core_ids=list(range(NCORES)))
    core_outs = [res.results[c]["out"] for c in range(NCORES)]
    bias_term = (bv @ Wo + bo).astype(np.float32)
    return combine_outputs(core_outs, bias_term)
